# revision 48
# baseline (speedup 1.0000x reference)
"""AConnect (nn_AConnect_82368882803074) Trainium2 kernel.

Reference computation:
    memW[b]    = W * Werr_bank[idx[b]]             [B, D_in, D_out]
    membias[b] = bias * Berr_bank[idx[b]]          [B, 1, D_out]
    Z[b]       = X[b] @ memW[b] + membias[b]       [B, D_out]

Strategy: data-parallel over the batch across 8 NeuronCores, with
duplicate-bank dedup. The host groups samples by bank index and packs the
banks onto cores ("slots"); each slot loads its bank matrix once and carries
up to M=4 samples as extra matmul columns. The host only moves data (gather,
transpose, zero-padding, output permutation); all arithmetic (W ⊙ E,
X @ (W ⊙ E), bias ⊙ Berr and the final add) runs on device.

Per core the device kernel streams K gathered 1 MB bank matrices from HBM,
casting f32->bf16 inside the (SWDGE) DMA, multiplies by W on VectorE in bf16
(2x mode), and contracts with the slot's 4 X-columns on TensorE (4 k-chunk
matmuls accumulating into a [4, 512] PSUM tile). VectorE then adds the
bias term while draining PSUM into a small output tile, which the (otherwise
idle) scalar-ring DMA writes straight to the slot's 4 output rows in DRAM.
Dummy matmuls on resident tiles keep the PE's HAM activity monitor busy so
real matmuls run at 2.4 GHz instead of 1.2.
"""

import numpy as np

B, D_IN, D_OUT, N_BANK, N_CORES = 256, 512, 512, 1000, 8
P = 128  # partitions
C = D_IN // P  # 4 k-chunks
M = 4  # samples per bank slot (max observed bank multiplicity is 3)

_CACHE = {}
last_exec_time_ns = None


def _build_nc(K):
    """Device graph for K bank-slots per core."""
    import concourse.mybir as mybir
    import concourse.tile as tile
    from concourse import bacc

    f32 = mybir.dt.float32
    bf16 = mybir.dt.bfloat16
    nc = bacc.Bacc()

    assert K % 2 == 0
    K2 = K // 2  # banks are loaded in pairs (2 MB per DMA)
    W2 = 2 * C * D_OUT  # free width of a paired tile
    R = K * M  # output rows, slot-major: row t*M + j = slot t, column j
    NB = (R + 31) // 32  # membias tiles of 32 rows each
    eg = nc.dram_tensor("eg", [K2, P, W2], f32, kind="ExternalInput")
    wt = nc.dram_tensor("wt", [P, W2], f32, kind="ExternalInput")
    xtt = nc.dram_tensor("xtt", [P, C * R], f32, kind="ExternalInput")
    bb = nc.dram_tensor("bb", [R, D_OUT], f32, kind="ExternalInput")
    beg = nc.dram_tensor("beg", [R, D_OUT], f32, kind="ExternalInput")
    out = nc.dram_tensor("out", [R, D_OUT], f32, kind="ExternalOutput")

    with tile.TileContext(nc) as tc:
        with (
            tc.tile_pool(name="const", bufs=1) as constp,
            tc.tile_pool(name="ep", bufs=6) as ep,
            tc.tile_pool(name="wep", bufs=4) as wep,
            tc.tile_pool(name="ps", bufs=6, space="PSUM") as psp,
            tc.tile_pool(name="scr", bufs=2) as scr,
            tc.tile_pool(name="mbp", bufs=16) as mbp,
            tc.tile_pool(name="outp", bufs=8) as outp,
        ):
            w_t = constp.tile([P, W2], f32)
            nc.sync.dma_start(w_t[:], wt[:])
            x_t = constp.tile([P, C * R], f32)
            nc.sync.dma_start(x_t[:], xtt[:])

            # membias = bias * Berr[bank], slot-major rows, in 32-row tiles
            mbts = []
            for q in range(NB):
                r0, r1 = q * 32, min((q + 1) * 32, R)
                bias_q = scr.tile([r1 - r0, D_OUT], f32, name=f"bias{q}", tag="bq")
                nc.sync.dma_start(bias_q[:], bb[r0:r1, :])
                berr_q = scr.tile([r1 - r0, D_OUT], f32, name=f"berr{q}", tag="eq")
                nc.sync.dma_start(berr_q[:], beg[r0:r1, :])
                mb_q = constp.tile([r1 - r0, D_OUT], f32, name=f"mb{q}")
                nc.vector.tensor_mul(mb_q[:], bias_q[:], berr_q[:])
                mbts.append(mb_q)

            # bf16 copies of the resident matmul operands
            w_b = constp.tile([P, W2], bf16)
            nc.vector.tensor_copy(w_b[:], w_t[:])
            x_b = constp.tile([P, C * R], bf16)
            nc.vector.tensor_copy(x_b[:], x_t[:])

            # Dummy matmuls on resident tiles keep the PE's HAM activity
            # monitor busy so real matmuls run at 2.4 GHz instead of 1.2.
            warm = psp.tile([M, D_OUT], f32, name="warm", bufs=1)

            def warm_mm():
                nc.tensor.matmul(
                    warm[:], x_b[:, 0:M], w_b[:, 0:D_OUT], start=True, stop=True
                )

            for _ in range(8):
                warm_mm()

            # membias rows per slot, moved to base-0 tiles upfront (the DVE
            # add needs equal base partitions; DMA does the relocation on
            # the otherwise idle scalar HWDGE ring)
            mb4s = []
            for t in range(K):
                r0 = t * M
                mb4 = mbp.tile([M, D_OUT], f32, name=f"mb4_{t}", tag="mb4")
                nc.scalar.dma_start(
                    mb4[:], mbts[r0 // 32][r0 % 32 : r0 % 32 + M, :]
                )
                mb4s.append(mb4)

            # pending: (slot, psum tile)
            pending = []

            def flush_one():
                t2, ps2 = pending.pop(0)
                osb = outp.tile([M, D_OUT], f32, name="osb", tag="osb")
                nc.vector.tensor_add(osb[:], ps2[:], mb4s[t2][:])
                nc.scalar.dma_start(out[t2 * M : (t2 + 1) * M, :], osb[:])

            for g in range(K2):
                # two banks per DMA; f32 -> bf16 cast happens inside the
                # (SWDGE) DMA
                eb = ep.tile([P, W2], bf16)
                nc.gpsimd.dma_start(eb[:], eg[g])
                we = wep.tile([P, W2], bf16)
                nc.vector.tensor_mul(we[:], eb[:], w_b[:])
                for u in range(2):
                    t = 2 * g + u
                    ps = psp.tile([M, D_OUT], f32)
                    for c in range(C):
                        nc.tensor.matmul(
                            ps[:],
                            x_b[:, (c * K + t) * M : (c * K + t) * M + M],
                            we[:, (u * C + c) * D_OUT : (u * C + c + 1) * D_OUT],
                            start=(c == 0),
                            stop=(c == C - 1),
                        )
                    warm_mm()
                    pending.append((t, ps))
                    # Drain the bias-add + store a few slots behind so the
                    # in-order DVE stream never waits on PE/PSUM.
                    if len(pending) > 3:
                        flush_one()
            while pending:
                flush_one()

    nc.compile()
    return nc


def _pack(idx):
    """Group samples by bank, pack banks onto cores.

    Returns (K, plan) where plan[c] is a list of (bank, [samples]) slots,
    each slot carrying at most M samples of one bank.
    """
    from collections import defaultdict

    groups = defaultdict(list)
    for s, b in enumerate(idx):
        groups[int(b)].append(s)
    # one slot per <=M samples of a bank
    slots = []
    for b, ss in groups.items():
        for i in range(0, len(ss), M):
            slots.append((b, ss[i : i + M]))
    slots.sort(key=lambda x: -len(x[1]))
    plan = [[] for _ in range(N_CORES)]
    for b, ss in slots:
        c = min(range(N_CORES), key=lambda c: len(plan[c]))
        plan[c].append((b, ss))
    K = max(len(p) for p in plan)
    K += K % 2  # banks are loaded in pairs
    return K, plan


def _install_trace_shim():
    """Register the axon NTFF profile hook bass_utils expects (the agent
    image lacks antenv.axon_hooks; the C ABI is in libaxon_pjrt.so)."""
    import contextlib
    import ctypes
    import sys
    import types

    if "antenv.axon_hooks" in sys.modules:
        return
    lib = ctypes.CDLL("/opt/axon/libaxon_pjrt.so")
    if not hasattr(lib, "axon_start_nrt_profile"):
        hook = None
    else:
        lib.axon_start_nrt_profile.argtypes = [
            ctypes.POINTER(ctypes.c_int64),
            ctypes.c_size_t,
        ]
        lib.axon_start_nrt_profile.restype = ctypes.c_int64
        lib.axon_stop_nrt_profile.argtypes = [ctypes.c_char_p]
        lib.axon_stop_nrt_profile.restype = ctypes.c_int64

        @contextlib.contextmanager
        def hook(output_dir, device_ids):
            import jax

            jax.devices()
            if device_ids:
                ids = (ctypes.c_int64 * len(device_ids))(*device_ids)
                rc = lib.axon_start_nrt_profile(ids, len(device_ids))
            else:
                rc = lib.axon_start_nrt_profile(None, 0)
            if rc != 0:
                raise RuntimeError(f"axon_start_nrt_profile rc={rc}")
            try:
                yield
            finally:
                n = lib.axon_stop_nrt_profile(str(output_dir).encode())
                print(f"ntff profile: {n} file(s) -> {output_dir}", file=sys.stderr)

    mod = types.ModuleType("antenv.axon_hooks")
    mod.get_axon_ntff_profile_hook = lambda: hook
    mod.set_axon_ntff_profile_hook = lambda h: None
    sys.modules["antenv.axon_hooks"] = mod


def kernel(X, W, bias, Werr_bank, Berr_bank, idx):
    global last_exec_time_ns
    import os

    from concourse.bass_utils import run_bass_kernel_spmd

    X = np.asarray(X, dtype=np.float32)
    W = np.asarray(W, dtype=np.float32)
    bias = np.asarray(bias, dtype=np.float32)
    Werr_bank = np.asarray(Werr_bank, dtype=np.float32)
    Berr_bank = np.asarray(Berr_bank, dtype=np.float32)
    idx = np.asarray(idx, dtype=np.int32)

    K, plan = _pack(idx)
    if ("nc", K) not in _CACHE:
        _CACHE[("nc", K)] = _build_nc(K)
    nc = _CACHE[("nc", K)]
    R = K * M

    # Host-side sharding / layout (pure data movement).
    wt1 = W.reshape(C, P, D_OUT).transpose(1, 0, 2).reshape(P, C * D_OUT)
    wt = np.ascontiguousarray(np.tile(wt1, (1, 2)))  # banks load in pairs
    bb = np.ascontiguousarray(np.broadcast_to(bias.reshape(1, D_OUT), (R, D_OUT)))

    in_maps = []
    row_of_sample = np.full(B, -1, dtype=np.int64)  # (core, row) flattened
    for c_id in range(N_CORES):
        slots = plan[c_id]
        banks = [b for b, _ in slots] + [0] * (K - len(slots))
        eg = Werr_bank[banks]  # [K, D_in, D_out]
        eg = (
            eg.reshape(K, C, P, D_OUT).transpose(0, 2, 1, 3).reshape(K, P, C * D_OUT)
        )
        # pair banks: eg2[g, p, u*2048 + f] = eg[2g + u, p, f]
        eg = np.ascontiguousarray(
            eg.reshape(K // 2, 2, P, C * D_OUT)
            .transpose(0, 2, 1, 3)
            .reshape(K // 2, P, 2 * C * D_OUT)
        )
        # X columns and output rows in slot-major order: row t*M + j
        xs = np.zeros((R, D_IN), dtype=np.float32)
        beg = np.zeros((R, D_OUT), dtype=np.float32)
        for t, (b, ss) in enumerate(slots):
            for j, s in enumerate(ss):
                xs[t * M + j] = X[s]
                beg[t * M + j] = Berr_bank[b, 0]
                row_of_sample[s] = c_id * R + t * M + j
        xtt = np.ascontiguousarray(
            xs.T.reshape(C, P, R).transpose(1, 0, 2).reshape(P, C * R)
        )
        in_maps.append({"eg": eg, "wt": wt, "xtt": xtt, "bb": bb, "beg": beg})
    assert (row_of_sample >= 0).all()

    trace = os.environ.get("BASS_KERNEL_TRACE") == "1"
    if trace:
        _install_trace_shim()
    res = run_bass_kernel_spmd(
        nc,
        in_maps,
        core_ids=list(range(N_CORES)),
        trace=trace,
        trace_cores=(
            list(range(N_CORES))
            if os.environ.get("BASS_KERNEL_TRACE_ALL") == "1"
            else [0]
        )
        if trace
        else None,
    )
    last_exec_time_ns = res.exec_time_ns
    allrows = np.concatenate([r["out"] for r in res.results], axis=0)  # [8*R, 512]
    return np.ascontiguousarray(allrows[row_of_sample])


# revision 49
# speedup vs baseline: 1.0627x; 1.0627x over previous
"""AConnect (nn_AConnect_82368882803074) Trainium2 kernel.

Reference computation:
    memW[b]    = W * Werr_bank[idx[b]]             [B, D_in, D_out]
    membias[b] = bias * Berr_bank[idx[b]]          [B, 1, D_out]
    Z[b]       = X[b] @ memW[b] + membias[b]       [B, D_out]

Strategy: data-parallel over the batch across 8 NeuronCores, with
duplicate-bank dedup. The host groups samples by bank index and packs the
banks onto cores ("slots"); each slot loads its bank matrix once and carries
up to M=4 samples as extra matmul columns. The host only moves data (gather,
transpose, zero-padding, output permutation); all arithmetic (W ⊙ E,
X @ (W ⊙ E), bias ⊙ Berr and the final add) runs on device.

Per core the device kernel streams K gathered 1 MB bank matrices from HBM,
casting f32->bf16 inside the (SWDGE) DMA, multiplies by W on VectorE in bf16
(2x mode), and contracts with the slot's 4 X-columns on TensorE (4 k-chunk
matmuls accumulating into a [4, 512] PSUM tile). VectorE then adds the
bias term while draining PSUM into a small output tile, which the (otherwise
idle) scalar-ring DMA writes straight to the slot's 4 output rows in DRAM.
Dummy matmuls on resident tiles keep the PE's HAM activity monitor busy so
real matmuls run at 2.4 GHz instead of 1.2.
"""

import numpy as np

B, D_IN, D_OUT, N_BANK, N_CORES = 256, 512, 512, 1000, 8
P = 128  # partitions
C = D_IN // P  # 4 k-chunks
M = 4  # samples per bank slot (max observed bank multiplicity is 3)

_CACHE = {}
last_exec_time_ns = None


def _build_nc(K):
    """Device graph for K bank-slots per core."""
    import concourse.mybir as mybir
    import concourse.tile as tile
    from concourse import bacc

    f32 = mybir.dt.float32
    bf16 = mybir.dt.bfloat16
    nc = bacc.Bacc()

    R = K * M  # output rows, slot-major: row t*M + j = slot t, column j
    NB = (R + 31) // 32  # membias tiles of 32 rows each
    eg = nc.dram_tensor("eg", [K, P, C * D_OUT], f32, kind="ExternalInput")
    wt = nc.dram_tensor("wt", [P, C * D_OUT], f32, kind="ExternalInput")
    xtt = nc.dram_tensor("xtt", [P, C * R], f32, kind="ExternalInput")
    bb = nc.dram_tensor("bb", [R, D_OUT], f32, kind="ExternalInput")
    beg = nc.dram_tensor("beg", [R, D_OUT], f32, kind="ExternalInput")
    out = nc.dram_tensor("out", [R, D_OUT], f32, kind="ExternalOutput")

    with tile.TileContext(nc) as tc:
        with (
            tc.tile_pool(name="const", bufs=1) as constp,
            tc.tile_pool(name="ep", bufs=10) as ep,
            tc.tile_pool(name="wep", bufs=6) as wep,
            tc.tile_pool(name="ps", bufs=6, space="PSUM") as psp,
            tc.tile_pool(name="scr", bufs=2) as scr,
            tc.tile_pool(name="mbp", bufs=K) as mbp,
            tc.tile_pool(name="outp", bufs=8) as outp,
        ):
            w_t = constp.tile([P, C * D_OUT], f32)
            nc.sync.dma_start(w_t[:], wt[:])
            x_t = constp.tile([P, C * R], f32)
            nc.sync.dma_start(x_t[:], xtt[:])

            # membias = bias * Berr[bank], slot-major rows, in 32-row tiles
            mbts = []
            for q in range(NB):
                r0, r1 = q * 32, min((q + 1) * 32, R)
                bias_q = scr.tile([r1 - r0, D_OUT], f32, name=f"bias{q}", tag="bq")
                nc.sync.dma_start(bias_q[:], bb[r0:r1, :])
                berr_q = scr.tile([r1 - r0, D_OUT], f32, name=f"berr{q}", tag="eq")
                nc.sync.dma_start(berr_q[:], beg[r0:r1, :])
                mb_q = constp.tile([r1 - r0, D_OUT], f32, name=f"mb{q}")
                nc.vector.tensor_mul(mb_q[:], bias_q[:], berr_q[:])
                mbts.append(mb_q)

            # bf16 copies of the resident matmul operands
            w_b = constp.tile([P, C * D_OUT], bf16)
            nc.vector.tensor_copy(w_b[:], w_t[:])
            x_b = constp.tile([P, C * R], bf16)
            nc.vector.tensor_copy(x_b[:], x_t[:])

            # Dummy matmuls on resident tiles keep the PE's HAM activity
            # monitor busy so real matmuls run at 2.4 GHz instead of 1.2.
            warm = psp.tile([M, D_OUT], f32, name="warm", bufs=1)

            def warm_mm():
                nc.tensor.matmul(
                    warm[:], x_b[:, 0:M], w_b[:, 0:D_OUT], start=True, stop=True
                )

            for _ in range(8):
                warm_mm()

            # membias rows per slot, moved to base-0 tiles upfront (the DVE
            # add needs equal base partitions; DMA does the relocation on
            # the otherwise idle scalar HWDGE ring)
            mb4s = []
            for t in range(K):
                r0 = t * M
                mb4 = mbp.tile([M, D_OUT], f32, name=f"mb4_{t}", tag="mb4")
                nc.scalar.dma_start(
                    mb4[:], mbts[r0 // 32][r0 % 32 : r0 % 32 + M, :]
                )
                mb4s.append(mb4)

            # pending: (slot, psum tile)
            pending = []

            def flush_one():
                t2, ps2 = pending.pop(0)
                osb = outp.tile([M, D_OUT], f32, name="osb", tag="osb")
                nc.vector.tensor_add(osb[:], ps2[:], mb4s[t2][:])
                nc.scalar.dma_start(out[t2 * M : (t2 + 1) * M, :], osb[:])

            for t in range(K):
                # f32 -> bf16 cast happens inside the (SWDGE) DMA
                eb = ep.tile([P, C * D_OUT], bf16)
                nc.gpsimd.dma_start(eb[:], eg[t])
                we = wep.tile([P, C * D_OUT], bf16)
                nc.vector.tensor_mul(we[:], eb[:], w_b[:])
                ps = psp.tile([M, D_OUT], f32)
                for c in range(C):
                    nc.tensor.matmul(
                        ps[:],
                        x_b[:, (c * K + t) * M : (c * K + t) * M + M],
                        we[:, c * D_OUT : (c + 1) * D_OUT],
                        start=(c == 0),
                        stop=(c == C - 1),
                    )
                warm_mm()
                pending.append((t, ps))
                # Drain the bias-add + store a few slots behind so the
                # in-order DVE stream never waits on PE/PSUM.
                if len(pending) > 2:
                    flush_one()
            while pending:
                flush_one()

    nc.compile()
    return nc


def _pack(idx):
    """Group samples by bank, pack banks onto cores.

    Returns (K, plan) where plan[c] is a list of (bank, [samples]) slots,
    each slot carrying at most M samples of one bank.
    """
    from collections import defaultdict

    groups = defaultdict(list)
    for s, b in enumerate(idx):
        groups[int(b)].append(s)
    # one slot per <=M samples of a bank
    slots = []
    for b, ss in groups.items():
        for i in range(0, len(ss), M):
            slots.append((b, ss[i : i + M]))
    slots.sort(key=lambda x: -len(x[1]))
    plan = [[] for _ in range(N_CORES)]
    for b, ss in slots:
        c = min(range(N_CORES), key=lambda c: len(plan[c]))
        plan[c].append((b, ss))
    K = max(len(p) for p in plan)
    return K, plan


def _install_trace_shim():
    """Register the axon NTFF profile hook bass_utils expects (the agent
    image lacks antenv.axon_hooks; the C ABI is in libaxon_pjrt.so)."""
    import contextlib
    import ctypes
    import sys
    import types

    if "antenv.axon_hooks" in sys.modules:
        return
    lib = ctypes.CDLL("/opt/axon/libaxon_pjrt.so")
    if not hasattr(lib, "axon_start_nrt_profile"):
        hook = None
    else:
        lib.axon_start_nrt_profile.argtypes = [
            ctypes.POINTER(ctypes.c_int64),
            ctypes.c_size_t,
        ]
        lib.axon_start_nrt_profile.restype = ctypes.c_int64
        lib.axon_stop_nrt_profile.argtypes = [ctypes.c_char_p]
        lib.axon_stop_nrt_profile.restype = ctypes.c_int64

        @contextlib.contextmanager
        def hook(output_dir, device_ids):
            import jax

            jax.devices()
            if device_ids:
                ids = (ctypes.c_int64 * len(device_ids))(*device_ids)
                rc = lib.axon_start_nrt_profile(ids, len(device_ids))
            else:
                rc = lib.axon_start_nrt_profile(None, 0)
            if rc != 0:
                raise RuntimeError(f"axon_start_nrt_profile rc={rc}")
            try:
                yield
            finally:
                n = lib.axon_stop_nrt_profile(str(output_dir).encode())
                print(f"ntff profile: {n} file(s) -> {output_dir}", file=sys.stderr)

    mod = types.ModuleType("antenv.axon_hooks")
    mod.get_axon_ntff_profile_hook = lambda: hook
    mod.set_axon_ntff_profile_hook = lambda h: None
    sys.modules["antenv.axon_hooks"] = mod


def kernel(X, W, bias, Werr_bank, Berr_bank, idx):
    global last_exec_time_ns
    import os

    from concourse.bass_utils import run_bass_kernel_spmd

    X = np.asarray(X, dtype=np.float32)
    W = np.asarray(W, dtype=np.float32)
    bias = np.asarray(bias, dtype=np.float32)
    Werr_bank = np.asarray(Werr_bank, dtype=np.float32)
    Berr_bank = np.asarray(Berr_bank, dtype=np.float32)
    idx = np.asarray(idx, dtype=np.int32)

    K, plan = _pack(idx)
    if ("nc", K) not in _CACHE:
        _CACHE[("nc", K)] = _build_nc(K)
    nc = _CACHE[("nc", K)]
    R = K * M

    # Host-side sharding / layout (pure data movement).
    wt = np.ascontiguousarray(
        W.reshape(C, P, D_OUT).transpose(1, 0, 2).reshape(P, C * D_OUT)
    )
    bb = np.ascontiguousarray(np.broadcast_to(bias.reshape(1, D_OUT), (R, D_OUT)))

    in_maps = []
    row_of_sample = np.full(B, -1, dtype=np.int64)  # (core, row) flattened
    for c_id in range(N_CORES):
        slots = plan[c_id]
        banks = [b for b, _ in slots] + [0] * (K - len(slots))
        eg = Werr_bank[banks]  # [K, D_in, D_out]
        eg = np.ascontiguousarray(
            eg.reshape(K, C, P, D_OUT).transpose(0, 2, 1, 3).reshape(K, P, C * D_OUT)
        )
        # X columns and output rows in slot-major order: row t*M + j
        xs = np.zeros((R, D_IN), dtype=np.float32)
        beg = np.zeros((R, D_OUT), dtype=np.float32)
        for t, (b, ss) in enumerate(slots):
            for j, s in enumerate(ss):
                xs[t * M + j] = X[s]
                beg[t * M + j] = Berr_bank[b, 0]
                row_of_sample[s] = c_id * R + t * M + j
        xtt = np.ascontiguousarray(
            xs.T.reshape(C, P, R).transpose(1, 0, 2).reshape(P, C * R)
        )
        in_maps.append({"eg": eg, "wt": wt, "xtt": xtt, "bb": bb, "beg": beg})
    assert (row_of_sample >= 0).all()

    trace = os.environ.get("BASS_KERNEL_TRACE") == "1"
    if trace:
        _install_trace_shim()
    res = run_bass_kernel_spmd(
        nc,
        in_maps,
        core_ids=list(range(N_CORES)),
        trace=trace,
        trace_cores=(
            list(range(N_CORES))
            if os.environ.get("BASS_KERNEL_TRACE_ALL") == "1"
            else [0]
        )
        if trace
        else None,
    )
    last_exec_time_ns = res.exec_time_ns
    allrows = np.concatenate([r["out"] for r in res.results], axis=0)  # [8*R, 512]
    return np.ascontiguousarray(allrows[row_of_sample])


# revision 50
# speedup vs baseline: 1.1523x; 1.0843x over previous
"""AConnect (nn_AConnect_82368882803074) Trainium2 kernel.

Reference computation:
    memW[b]    = W * Werr_bank[idx[b]]             [B, D_in, D_out]
    membias[b] = bias * Berr_bank[idx[b]]          [B, 1, D_out]
    Z[b]       = X[b] @ memW[b] + membias[b]       [B, D_out]

Strategy: data-parallel over the batch across 8 NeuronCores, with
duplicate-bank dedup. The host groups samples by bank index and packs the
banks onto cores ("slots"); each slot loads its bank matrix once and carries
up to M=4 samples as extra matmul columns. The host only moves data (gather,
transpose, zero-padding, output permutation); all arithmetic (W ⊙ E,
X @ (W ⊙ E), bias ⊙ Berr and the final add) runs on device.

Per core the device kernel streams K gathered 1 MB bank matrices from HBM,
casting f32->bf16 inside the (SWDGE) DMA, multiplies by W on VectorE in bf16
(2x mode), and contracts with the slot's 4 X-columns on TensorE (4 k-chunk
matmuls accumulating into a [4, 512] PSUM tile). VectorE then adds the
bias term while draining PSUM into a small output tile, which the (otherwise
idle) scalar-ring DMA writes straight to the slot's 4 output rows in DRAM.
Dummy matmuls on resident tiles keep the PE's HAM activity monitor busy so
real matmuls run at 2.4 GHz instead of 1.2.
"""

import numpy as np

B, D_IN, D_OUT, N_BANK, N_CORES = 256, 512, 512, 1000, 8
P = 128  # partitions
C = D_IN // P  # 4 k-chunks
M = 4  # samples per bank slot (max observed bank multiplicity is 3)

_CACHE = {}
last_exec_time_ns = None


def _build_nc(K):
    """Device graph for K bank-slots per core."""
    import concourse.mybir as mybir
    import concourse.tile as tile
    from concourse import bacc

    f32 = mybir.dt.float32
    bf16 = mybir.dt.bfloat16
    nc = bacc.Bacc()

    R = K * M  # output rows, slot-major: row t*M + j = slot t, column j
    NB = (R + 31) // 32  # membias tiles of 32 rows each
    eg = nc.dram_tensor("eg", [K, P, C * D_OUT], f32, kind="ExternalInput")
    wt = nc.dram_tensor("wt", [P, C * D_OUT], f32, kind="ExternalInput")
    xtt = nc.dram_tensor("xtt", [P, C * R], f32, kind="ExternalInput")
    bb = nc.dram_tensor("bb", [R, D_OUT], f32, kind="ExternalInput")
    beg = nc.dram_tensor("beg", [R, D_OUT], f32, kind="ExternalInput")
    out = nc.dram_tensor("out", [R, D_OUT], f32, kind="ExternalOutput")

    with tile.TileContext(nc) as tc:
        with (
            tc.tile_pool(name="const", bufs=1) as constp,
            tc.tile_pool(name="ep", bufs=12) as ep,
            tc.tile_pool(name="wep", bufs=6) as wep,
            tc.tile_pool(name="ps", bufs=7, space="PSUM") as psp,
            tc.tile_pool(name="scr", bufs=2) as scr,
            tc.tile_pool(name="mbp", bufs=K) as mbp,
            tc.tile_pool(name="outp", bufs=8) as outp,
        ):
            w_t = constp.tile([P, C * D_OUT], f32)
            nc.sync.dma_start(w_t[:], wt[:])
            x_t = constp.tile([P, C * R], f32)
            nc.sync.dma_start(x_t[:], xtt[:])

            # membias = bias * Berr[bank], slot-major rows, in 32-row tiles
            mbts = []
            for q in range(NB):
                r0, r1 = q * 32, min((q + 1) * 32, R)
                bias_q = scr.tile([r1 - r0, D_OUT], f32, name=f"bias{q}", tag="bq")
                nc.sync.dma_start(bias_q[:], bb[r0:r1, :])
                berr_q = scr.tile([r1 - r0, D_OUT], f32, name=f"berr{q}", tag="eq")
                nc.sync.dma_start(berr_q[:], beg[r0:r1, :])
                mb_q = constp.tile([r1 - r0, D_OUT], f32, name=f"mb{q}")
                nc.vector.tensor_mul(mb_q[:], bias_q[:], berr_q[:])
                mbts.append(mb_q)

            # bf16 copies of the resident matmul operands
            w_b = constp.tile([P, C * D_OUT], bf16)
            nc.vector.tensor_copy(w_b[:], w_t[:])
            x_b = constp.tile([P, C * R], bf16)
            nc.vector.tensor_copy(x_b[:], x_t[:])

            # Dummy matmuls on resident tiles keep the PE's HAM activity
            # monitor busy so real matmuls run at 2.4 GHz instead of 1.2.
            warm = psp.tile([M, D_OUT], f32, name="warm", bufs=1)

            def warm_mm():
                nc.tensor.matmul(
                    warm[:], x_b[:, 0:M], w_b[:, 0:D_OUT], start=True, stop=True
                )

            for _ in range(8):
                warm_mm()

            # membias rows per slot, moved to base-0 tiles upfront (the DVE
            # add needs equal base partitions; DMA does the relocation on
            # the otherwise idle scalar HWDGE ring)
            mb4s = []
            for t in range(K):
                r0 = t * M
                mb4 = mbp.tile([M, D_OUT], f32, name=f"mb4_{t}", tag="mb4")
                nc.scalar.dma_start(
                    mb4[:], mbts[r0 // 32][r0 % 32 : r0 % 32 + M, :]
                )
                mb4s.append(mb4)

            # pending: (slot, psum tile)
            pending = []

            def flush_one():
                t2, ps2 = pending.pop(0)
                osb = outp.tile([M, D_OUT], f32, name="osb", tag="osb")
                nc.vector.tensor_add(osb[:], ps2[:], mb4s[t2][:])
                nc.scalar.dma_start(out[t2 * M : (t2 + 1) * M, :], osb[:])

            for t in range(K):
                # f32 -> bf16 cast happens inside the (SWDGE) DMA
                eb = ep.tile([P, C * D_OUT], bf16)
                nc.gpsimd.dma_start(eb[:], eg[t])
                we = wep.tile([P, C * D_OUT], bf16)
                nc.vector.tensor_mul(we[:], eb[:], w_b[:])
                ps = psp.tile([M, D_OUT], f32)
                for c in range(C):
                    nc.tensor.matmul(
                        ps[:],
                        x_b[:, (c * K + t) * M : (c * K + t) * M + M],
                        we[:, c * D_OUT : (c + 1) * D_OUT],
                        start=(c == 0),
                        stop=(c == C - 1),
                    )
                warm_mm()
                pending.append((t, ps))
                # Drain the bias-add + store a few slots behind so the
                # in-order DVE stream never waits on PE/PSUM.
                if len(pending) > 2:
                    flush_one()
            while pending:
                flush_one()

    nc.compile()
    return nc


def _pack(idx):
    """Group samples by bank, pack banks onto cores.

    Returns (K, plan) where plan[c] is a list of (bank, [samples]) slots,
    each slot carrying at most M samples of one bank.
    """
    from collections import defaultdict

    groups = defaultdict(list)
    for s, b in enumerate(idx):
        groups[int(b)].append(s)
    # one slot per <=M samples of a bank
    slots = []
    for b, ss in groups.items():
        for i in range(0, len(ss), M):
            slots.append((b, ss[i : i + M]))
    slots.sort(key=lambda x: -len(x[1]))
    plan = [[] for _ in range(N_CORES)]
    for b, ss in slots:
        c = min(range(N_CORES), key=lambda c: len(plan[c]))
        plan[c].append((b, ss))
    K = max(len(p) for p in plan)
    return K, plan


def _install_trace_shim():
    """Register the axon NTFF profile hook bass_utils expects (the agent
    image lacks antenv.axon_hooks; the C ABI is in libaxon_pjrt.so)."""
    import contextlib
    import ctypes
    import sys
    import types

    if "antenv.axon_hooks" in sys.modules:
        return
    lib = ctypes.CDLL("/opt/axon/libaxon_pjrt.so")
    if not hasattr(lib, "axon_start_nrt_profile"):
        hook = None
    else:
        lib.axon_start_nrt_profile.argtypes = [
            ctypes.POINTER(ctypes.c_int64),
            ctypes.c_size_t,
        ]
        lib.axon_start_nrt_profile.restype = ctypes.c_int64
        lib.axon_stop_nrt_profile.argtypes = [ctypes.c_char_p]
        lib.axon_stop_nrt_profile.restype = ctypes.c_int64

        @contextlib.contextmanager
        def hook(output_dir, device_ids):
            import jax

            jax.devices()
            if device_ids:
                ids = (ctypes.c_int64 * len(device_ids))(*device_ids)
                rc = lib.axon_start_nrt_profile(ids, len(device_ids))
            else:
                rc = lib.axon_start_nrt_profile(None, 0)
            if rc != 0:
                raise RuntimeError(f"axon_start_nrt_profile rc={rc}")
            try:
                yield
            finally:
                n = lib.axon_stop_nrt_profile(str(output_dir).encode())
                print(f"ntff profile: {n} file(s) -> {output_dir}", file=sys.stderr)

    mod = types.ModuleType("antenv.axon_hooks")
    mod.get_axon_ntff_profile_hook = lambda: hook
    mod.set_axon_ntff_profile_hook = lambda h: None
    sys.modules["antenv.axon_hooks"] = mod


def kernel(X, W, bias, Werr_bank, Berr_bank, idx):
    global last_exec_time_ns
    import os

    from concourse.bass_utils import run_bass_kernel_spmd

    X = np.asarray(X, dtype=np.float32)
    W = np.asarray(W, dtype=np.float32)
    bias = np.asarray(bias, dtype=np.float32)
    Werr_bank = np.asarray(Werr_bank, dtype=np.float32)
    Berr_bank = np.asarray(Berr_bank, dtype=np.float32)
    idx = np.asarray(idx, dtype=np.int32)

    K, plan = _pack(idx)
    if ("nc", K) not in _CACHE:
        _CACHE[("nc", K)] = _build_nc(K)
    nc = _CACHE[("nc", K)]
    R = K * M

    # Host-side sharding / layout (pure data movement).
    wt = np.ascontiguousarray(
        W.reshape(C, P, D_OUT).transpose(1, 0, 2).reshape(P, C * D_OUT)
    )
    bb = np.ascontiguousarray(np.broadcast_to(bias.reshape(1, D_OUT), (R, D_OUT)))

    in_maps = []
    row_of_sample = np.full(B, -1, dtype=np.int64)  # (core, row) flattened
    for c_id in range(N_CORES):
        slots = plan[c_id]
        banks = [b for b, _ in slots] + [0] * (K - len(slots))
        eg = Werr_bank[banks]  # [K, D_in, D_out]
        eg = np.ascontiguousarray(
            eg.reshape(K, C, P, D_OUT).transpose(0, 2, 1, 3).reshape(K, P, C * D_OUT)
        )
        # X columns and output rows in slot-major order: row t*M + j
        xs = np.zeros((R, D_IN), dtype=np.float32)
        beg = np.zeros((R, D_OUT), dtype=np.float32)
        for t, (b, ss) in enumerate(slots):
            for j, s in enumerate(ss):
                xs[t * M + j] = X[s]
                beg[t * M + j] = Berr_bank[b, 0]
                row_of_sample[s] = c_id * R + t * M + j
        xtt = np.ascontiguousarray(
            xs.T.reshape(C, P, R).transpose(1, 0, 2).reshape(P, C * R)
        )
        in_maps.append({"eg": eg, "wt": wt, "xtt": xtt, "bb": bb, "beg": beg})
    assert (row_of_sample >= 0).all()

    trace = os.environ.get("BASS_KERNEL_TRACE") == "1"
    if trace:
        _install_trace_shim()
    res = run_bass_kernel_spmd(
        nc,
        in_maps,
        core_ids=list(range(N_CORES)),
        trace=trace,
        trace_cores=(
            list(range(N_CORES))
            if os.environ.get("BASS_KERNEL_TRACE_ALL") == "1"
            else [0]
        )
        if trace
        else None,
    )
    last_exec_time_ns = res.exec_time_ns
    allrows = np.concatenate([r["out"] for r in res.results], axis=0)  # [8*R, 512]
    return np.ascontiguousarray(allrows[row_of_sample])
